# revision 21
# baseline (speedup 1.0000x reference)
"""Bahdanau additive attention kernel for Trainium2 (8 NeuronCores, SPMD).

Problem (hardcoded): B=32, Tq=4, S=2048, H=1024, 2H=2048, fp32 inputs.
  q  = query[:, -1, :]                      [B, H]
  k  = transpose(keys, (1, 0, 2))           [B, S, 2H]
  wq = q @ Wa_w.T + Wa_b                    [B, H]
  uk = k @ Ua_w.T + Ua_b                    [B, S, H]
  sc = tanh(wq[:, None, :] + uk) @ Va_w.T   [B, S]   (+ Va_b, which softmax cancels)
  w  = softmax(sc, axis=-1)                 [B, S]
  ctx = w @ k                               [B, 2H]
  returns (ctx [B,1,2H], w [B,1,S])

Sharding: data-parallel over batch. 8 cores x 4 batches each; weights
replicated; no cross-core communication.

Per-core dataflow (all matmuls bf16 with fp32 PSUM accumulation):
  - Ua/Wa are cast f32->bf16 (DRAM->DRAM SWDGE) then read back transposed via
    the DMA xbar transpose so the contraction dim lands on partitions.
  - keys strips [128, 2H] are cast-loaded f32->bf16 (kept in SBUF for the
    context matmul), stored to a DRAM scratch, and read back transposed
    ([d=128, s=512] tiles) for the big uk matmul.
  - ukT tiles [h=128, s=512] accumulate in PSUM; ScalarE applies
    tanh(. + bias[h]) where bias = wq[b] + Wa_b + Ua_b folded per-partition.
  - scores via PE with Va columns as the 1-wide stationary operand.
  - softmax without max-subtraction (scores are O(1)); exp on ScalarE with
    free-dim accumulate for the denominator.
  - context via PE with normalized-late weights columns (DMA-xbar transposed
    through DRAM) against the cached bf16 keys strips.
"""

import numpy as np

B, TQ, S, H = 32, 4, 2048, 1024
D2 = 2 * H
NCORES = 8
BPC = B // NCORES  # batches per core

_CACHE = {}


def _build(s=S, h=H, bpc=BPC, schunk=512):
    """Build the per-core Bass module. Parameterized so a scaled-down config
    can run in CoreSim; the shipped kernel uses the defaults."""
    from contextlib import ExitStack

    import concourse.bacc as bacc
    import concourse.bass as bass
    import concourse.mybir as mybir
    import concourse.tile as tile
    from concourse.masks import make_identity

    fp32 = mybir.dt.float32
    bf16 = mybir.dt.bfloat16
    AF = mybir.ActivationFunctionType
    d2 = 2 * h
    SD = d2 // 128        # contraction strips for uk (d on partitions)
    SM = h // 128         # h tiles (uk output partitions / Va strips)
    SJ = h // 128         # contraction strips for wq
    NCH = s // schunk     # score chunks per batch
    SPC = schunk // 128   # keys strips per chunk
    NDC = d2 // 512       # context output chunks
    NST = s // 128        # keys strips per batch

    nc = bacc.Bacc(
        "TRN2", target_bir_lowering=False, enable_partition_id=False
    )

    q_in = nc.dram_tensor("q", [bpc, h], fp32, kind="ExternalInput").ap()
    keys_in = nc.dram_tensor("keys", [s, bpc, d2], fp32, kind="ExternalInput").ap()
    wa_in = nc.dram_tensor("wa", [h, h], fp32, kind="ExternalInput").ap()
    wab_in = nc.dram_tensor("wab", [1, h], fp32, kind="ExternalInput").ap()
    ua_in = nc.dram_tensor("ua", [h, d2], fp32, kind="ExternalInput").ap()
    uab_in = nc.dram_tensor("uab", [1, h], fp32, kind="ExternalInput").ap()
    va_in = nc.dram_tensor("va", [1, h], fp32, kind="ExternalInput").ap()
    ctx_out = nc.dram_tensor("ctx", [bpc, d2], fp32, kind="ExternalOutput").ap()
    w_out = nc.dram_tensor("wts", [bpc, s], fp32, kind="ExternalOutput").ap()

    with tile.TileContext(nc) as tc:
        with ExitStack() as ctx:
            consts = ctx.enter_context(tc.tile_pool(name="consts", bufs=1))
            dram1 = ctx.enter_context(tc.tile_pool(name="dram1", bufs=1, space="DRAM"))
            dram_kn = ctx.enter_context(
                tc.tile_pool(name="dram_kn", bufs=2, space="DRAM")
            )
            kcache = ctx.enter_context(
                tc.tile_pool(name="kcache", bufs=3 * SPC)
            )
            ktp = ctx.enter_context(
                tc.tile_pool(name="ktp", bufs=2 * SD + 4)
            )
            tp = ctx.enter_context(tc.tile_pool(name="tp", bufs=SM + 2))
            rows = ctx.enter_context(tc.tile_pool(name="rows", bufs=2))
            ps_setup = ctx.enter_context(
                tc.tile_pool(name="ps_setup", bufs=2, space="PSUM")
            )
            ps_uk = ctx.enter_context(tc.tile_pool(name="ps_uk", bufs=2, space="PSUM"))
            ps_sc = ctx.enter_context(tc.tile_pool(name="ps_sc", bufs=2, space="PSUM"))
            ps_cx = ctx.enter_context(tc.tile_pool(name="ps_cx", bufs=2, space="PSUM"))

            # ---------------- one-time setup ----------------
            # Identity matrices + small vector loads first: the wq chain
            # (waT -> qT -> wq matmuls -> bias_cols) is the deepest setup
            # dependency, so its producers go to the front of every queue.
            ident = consts.tile([128, 128], bf16)
            make_identity(nc, ident)
            ident_f32 = consts.tile([128, 128], fp32)
            make_identity(nc, ident_f32)

            # Small vectors (cast to bf16 on load where needed).
            q_bf = consts.tile([bpc, h], bf16)
            nc.gpsimd.dma_start(out=q_bf, in_=q_in)
            va_bf = consts.tile([1, h], bf16)
            nc.gpsimd.dma_start(out=va_bf, in_=va_in)
            uab_row = consts.tile([1, h], bf16)
            nc.gpsimd.dma_start(out=uab_row, in_=uab_in)
            wab_row = consts.tile([1, h], bf16)
            nc.gpsimd.dma_start(out=wab_row, in_=wab_in)
            # combined additive bias row (Wa_b + Ua_b), bf16 for the K=1 matmul
            comb_bf = consts.tile([1, h], bf16)
            nc.vector.tensor_tensor(
                out=comb_bf, in0=uab_row, in1=wab_row, op=mybir.AluOpType.add
            )
            ones_bf = consts.tile([1, bpc], bf16)
            nc.vector.memset(ones_bf, 1.0)

            # bf16 copies of Ua/Wa staged to DRAM (one contiguous SWDGE cast
            # each — strided per-strip casts are descriptor-bound and slow),
            # then read back transposed per 128-column strip via the xbar.
            wa_bf = dram1.tile([h, h], bf16)
            nc.gpsimd.dma_start(out=wa_bf, in_=wa_in)
            ua_bf = dram1.tile([h, d2], bf16)
            nc.gpsimd.dma_start(out=ua_bf, in_=ua_in)

            # keys pipeline for one chunk: strips are cast-loaded f32->bf16
            # into SBUF (reused by the context matmul), staged to DRAM in the
            # natural layout, and read back transposed as kT tiles.
            def load_chunk(knat, b, c):
                strips = []
                for i in range(SPC):
                    si = c * SPC + i
                    ks = kcache.tile([128, d2], bf16, tag="ks", name=f"ks_{b}_{si}")
                    nc.gpsimd.dma_start(
                        out=ks, in_=keys_in[si * 128 : (si + 1) * 128, b, :]
                    )
                    nc.gpsimd.dma_start(
                        out=knat[si * 128 : (si + 1) * 128, :], in_=ks
                    )
                    strips.append(ks)
                kts = []
                for d in range(SD):
                    kt = ktp.tile(
                        [128, schunk], bf16, tag="kt", name=f"kt_{b}_{c}_{d}"
                    )
                    nc.sync.dma_start(
                        out=kt,
                        in_=knat[
                            c * schunk : (c + 1) * schunk, d * 128 : (d + 1) * 128
                        ],
                        transpose=True,
                    )
                    kts.append(kt)
                return strips, kts

            # Transposed weight strips. Sync-queue order matters: waT (gates
            # the wq chain), then batch-0 chunk-0's kT tiles, then uaT.
            waT = consts.tile([128, SJ, h], bf16)
            for j in range(SJ):
                nc.sync.dma_start(
                    out=waT[:, j, :],
                    in_=wa_bf[:, j * 128 : (j + 1) * 128],
                    transpose=True,
                )
            knat0 = dram_kn.tile([s, d2], bf16, tag="knat", name="knat_b0")
            chunk00 = load_chunk(knat0, 0, 0)
            # uaT[:, d, :] = Ua[:, 128d:128d+128].T  -> [dpart=128, h]
            uaT = consts.tile([128, SD, h], bf16)
            for d in range(SD):
                nc.sync.dma_start(
                    out=uaT[:, d, :],
                    in_=ua_bf[:, d * 128 : (d + 1) * 128],
                    transpose=True,
                )

            # qT strips [j=128, bpc] via PE transpose of q_bf
            qT = consts.tile([128, SJ, bpc], bf16)
            for j in range(SJ):
                ptr = ps_setup.tile([128, bpc], bf16, tag="setup")
                nc.tensor.transpose(
                    out=ptr,
                    in_=q_bf[:, j * 128 : (j + 1) * 128],
                    identity=ident[:bpc, :bpc],
                )
                nc.vector.tensor_copy(out=qT[:, j, :], in_=ptr)

            # Va columns [h=128, SM] via PE transpose of the bf16 row
            va_cols = consts.tile([128, SM], bf16)
            for m in range(SM):
                vtr = ps_setup.tile([128, 1], bf16, tag="setup")
                nc.tensor.transpose(
                    out=vtr,
                    in_=va_bf[:1, m * 128 : (m + 1) * 128],
                    identity=ident[:1, :1],
                )
                nc.vector.tensor_copy(out=va_cols[:, m : m + 1], in_=vtr)

            # bias_cols[:, m, b] = (Wa q_b)[128m:128m+128] + Wa_b + Ua_b  (fp32)
            bias_cols = consts.tile([128, SM, bpc], fp32)
            for m in range(SM):
                pw = ps_setup.tile([128, bpc], fp32, tag="setup")
                for j in range(SJ):
                    nc.tensor.matmul(
                        out=pw,
                        lhsT=waT[:, j, m * 128 : (m + 1) * 128],
                        rhs=qT[:, j, :],
                        start=(j == 0),
                        stop=False,
                    )
                nc.tensor.matmul(
                    out=pw,
                    lhsT=comb_bf[:1, m * 128 : (m + 1) * 128],
                    rhs=ones_bf,
                    start=False,
                    stop=True,
                )
                nc.vector.tensor_copy(out=bias_cols[:, m, :], in_=pw)

            # ---------------- main loop over batches ----------------
            for b in range(bpc):
                if b == 0:
                    knat = knat0
                else:
                    knat = dram_kn.tile(
                        [s, d2], bf16, tag="knat", name=f"knat_b{b}"
                    )
                exp_row = rows.tile([1, s], fp32, tag="exp_row")
                tparts = rows.tile([1, NCH], fp32, tag="tparts")
                ecols = rows.tile([128, NST], bf16, tag="ecols")
                ctx_acc = rows.tile([1, d2], fp32, tag="ctx_acc")
                for c in range(NCH):
                    if b == 0 and c == 0:
                        strips, kts = chunk00
                    else:
                        strips, kts = load_chunk(knat, b, c)
                    # ukT tiles + tanh; score matmuls are deferred until all
                    # tanh tiles exist so the in-order PE queue never waits
                    # on the Scalar engine mid-chunk
                    psc = ps_sc.tile([1, schunk], fp32, tag="psc")
                    ts_list = []
                    for m in range(SM):
                        puk = ps_uk.tile([128, schunk], fp32, tag="puk")
                        for d in range(SD):
                            nc.tensor.matmul(
                                out=puk,
                                lhsT=uaT[:, d, m * 128 : (m + 1) * 128],
                                rhs=kts[d],
                                start=(d == 0),
                                stop=(d == SD - 1),
                            )
                        t_sb = tp.tile([128, schunk], bf16, tag="t")
                        nc.scalar.activation(
                            out=t_sb,
                            in_=puk,
                            func=AF.Tanh,
                            bias=bias_cols[:, m, b : b + 1],
                            scale=1.0,
                        )
                        ts_list.append(t_sb)
                    for m in range(SM):
                        nc.tensor.matmul(
                            out=psc,
                            lhsT=va_cols[:, m : m + 1],
                            rhs=ts_list[m],
                            start=(m == 0),
                            stop=(m == SM - 1),
                        )
                    # exp row chunk (no max subtraction; scores are O(1)) and
                    # the chunk's softmax partial sum
                    nc.scalar.activation(
                        out=exp_row[:, c * schunk : (c + 1) * schunk],
                        in_=psc,
                        func=AF.Exp,
                        accum_out=tparts[:, c : c + 1],
                    )
                    # transpose this chunk's scores into columns on PE (tiny)
                    # and exp them -> unnormalized weight columns for context
                    scsb = rows.tile([1, schunk], fp32, tag="scsb")
                    nc.vector.tensor_copy(out=scsb, in_=psc)
                    pscT = ps_setup.tile([128, SPC], fp32, tag="setup")
                    for g in range(SPC):
                        nc.tensor.transpose(
                            out=pscT[:, g : g + 1],
                            in_=scsb[:1, g * 128 : (g + 1) * 128],
                            identity=ident_f32[:1, :1],
                        )
                    nc.scalar.activation(
                        out=ecols[:, c * SPC : (c + 1) * SPC],
                        in_=pscT,
                        func=AF.Exp,
                    )
                    # context partial for this chunk's strips (normalized at
                    # the end of the batch): ctx += sum_si e[si] * k[si, :]
                    for jd in range(NDC):
                        pcx = ps_cx.tile([1, 512], fp32, tag="pcx")
                        for i in range(SPC):
                            nc.tensor.matmul(
                                out=pcx,
                                lhsT=ecols[:, c * SPC + i : c * SPC + i + 1],
                                rhs=strips[i][:, jd * 512 : (jd + 1) * 512],
                                start=(i == 0),
                                stop=(i == SPC - 1),
                            )
                        if c == 0:
                            nc.vector.tensor_copy(
                                out=ctx_acc[:, jd * 512 : (jd + 1) * 512], in_=pcx
                            )
                        else:
                            nc.vector.tensor_add(
                                out=ctx_acc[:, jd * 512 : (jd + 1) * 512],
                                in0=ctx_acc[:, jd * 512 : (jd + 1) * 512],
                                in1=pcx,
                            )
                # softmax denominator; normalize weights + context, write out
                tsum = rows.tile([1, 1], fp32, tag="tsum")
                nc.vector.reduce_sum(
                    out=tsum, in_=tparts, axis=mybir.AxisListType.X
                )
                invt = rows.tile([1, 1], fp32, tag="invt")
                nc.vector.reciprocal(out=invt, in_=tsum)
                nc.vector.tensor_scalar_mul(out=exp_row, in0=exp_row, scalar1=invt)
                nc.scalar.dma_start(out=w_out[b : b + 1, :], in_=exp_row)
                nc.vector.tensor_scalar_mul(out=ctx_acc, in0=ctx_acc, scalar1=invt)
                nc.scalar.dma_start(out=ctx_out[b : b + 1, :], in_=ctx_acc)

    nc.compile()
    return nc


def _get_nc():
    if "nc" not in _CACHE:
        _CACHE["nc"] = _build()
    return _CACHE["nc"]


def _make_in_maps(inputs):
    q_last = np.ascontiguousarray(
        np.asarray(inputs["query"], dtype=np.float32)[:, -1, :]
    )  # [B, H]
    keys = np.asarray(inputs["keys"], dtype=np.float32)  # [S, B, 2H]
    wa = np.ascontiguousarray(np.asarray(inputs["Wa_w"], dtype=np.float32))
    wab = np.asarray(inputs["Wa_b"], dtype=np.float32).reshape(1, H)
    ua = np.ascontiguousarray(np.asarray(inputs["Ua_w"], dtype=np.float32))
    uab = np.asarray(inputs["Ua_b"], dtype=np.float32).reshape(1, H)
    va = np.ascontiguousarray(np.asarray(inputs["Va_w"], dtype=np.float32)).reshape(
        1, H
    )
    in_maps = []
    for c in range(NCORES):
        b0 = c * BPC
        in_maps.append(
            {
                "q": np.ascontiguousarray(q_last[b0 : b0 + BPC]),
                "keys": np.ascontiguousarray(keys[:, b0 : b0 + BPC, :]),
                "wa": wa,
                "wab": wab,
                "ua": ua,
                "uab": uab,
                "va": va,
            }
        )
    return in_maps


def run(inputs, trace=False, **kwargs):
    """Run on all 8 cores; returns ((context, weights), BassKernelResults)."""
    from concourse.bass_utils import run_bass_kernel_spmd

    nc = _get_nc()
    in_maps = _make_in_maps(inputs)
    res = run_bass_kernel_spmd(
        nc, in_maps, core_ids=list(range(NCORES)), trace=trace, **kwargs
    )
    context = np.empty((B, 1, D2), dtype=np.float32)
    weights = np.empty((B, 1, S), dtype=np.float32)
    for c in range(NCORES):
        b0 = c * BPC
        context[b0 : b0 + BPC, 0, :] = res.results[c]["ctx"]
        weights[b0 : b0 + BPC, 0, :] = res.results[c]["wts"]
    return (context, weights), res


def kernel(**inputs):
    out, _ = run(inputs)
    return out
